# revision 12
# baseline (speedup 1.0000x reference)
"""Trainium2 Bass kernel for nn_Body_GNN (gnn_message_passing).

Sharding: nodes are split 512/core across 8 NeuronCores. Per core:
  - grouped conv1/conv2 (per-node weights) run on the tensor engine as
    diagonal-matrix matmuls with tap accumulation in PSUM,
  - xw = h @ W_gcn and lin1 = X @ lin1_W as standard matmuls
    (X pre-transposed on host, h transposed on the PE),
  - xw shards exchanged with AllGather,
  - GCN aggregate as dense A_shard @ xw_full (A built on host from
    edge_index: symmetric norms + self-loop diagonal),
  - GraphNorm (stats over batch) + elu on vector/scalar engines,
  - node-mean pool via ones-vector matmul + AllReduce.
"""
import sys

sys.path.insert(0, "/opt/trn_rl_repo")

import numpy as np
import concourse.bass as bass
import concourse.mybir as mybir
import concourse.tile as tile
from concourse.vector_clock import ScopedClock
from concourse.bass_utils import run_bass_kernel_spmd

# ---------------------------------------------------------------------------
# Workaround: CoreV3 codegen in this toolchain rejects >1 sync-wait on the
# TileContext tail Drain; split the waits across sync-engine NOPs instead.
_MAXW = 1


def _patched_drain_and_barrier(self, tick_clock, wait_clock):
    nc = self.nc
    probe = nc.sync.nop(nofuse=True)
    wait_clock.add_sem_waits(probe.ins, ScopedClock({None: tick_clock.global_clock}))
    si = probe.ins.sync_info
    waits = list(si.on_wait) if si and si.on_wait else []
    if len(waits) > _MAXW:
        si.on_wait = waits[:_MAXW]
        rest = waits[_MAXW:]
        for i in range(0, len(rest), _MAXW):
            n2 = nc.sync.nop(nofuse=True)
            if n2.ins.sync_info is None:
                n2.ins.sync_info = mybir.SyncInfo(
                    on_wait=rest[i : i + _MAXW], on_update=[]
                )
            else:
                n2.ins.sync_info.on_wait = rest[i : i + _MAXW]
    nc.sync.drain()
    nc.all_engine_barrier()
    assert self.sems is not None
    popped = nc._tile_sem_poison_stack.pop()
    assert popped is self._sem_poison
    nc.clear_and_free_semaphores(list(self.sems.allocated().values()))
    nc.all_engine_barrier()


tile.TileContext._drain_and_barrier = _patched_drain_and_barrier

# ---------------------------------------------------------------------------
B, N, L, KW, C = 4, 4096, 240, 3, 256
L1 = L - KW + 1  # 238, conv1 output length
L2 = L1 - KW + 1  # 236, conv2 output length
NCORES = 8
NSH = N // NCORES  # 512 nodes per core
NT = NSH // 128  # 4 node tiles per core
ST = N // 128  # 32 source k-tiles for the aggregate
GN_EPS = 1e-5

AF = mybir.ActivationFunctionType
OP = mybir.AluOpType
F32 = mybir.dt.float32

_built = None
DEBUG = False

# This walrus build rejects instructions carrying more than _MAX_ISA_WAITS
# sync waits. Hoist the excess onto same-engine NoOps placed just before.
_MAX_ISA_WAITS = 1


def _split_sync_waits(nc, maxw=_MAX_ISA_WAITS):
    for f in nc.m.functions:
        for bb in f.blocks:
            new_insts = []
            for inst in bb.instructions:
                si = inst.sync_info
                if si is not None and si.on_wait and len(si.on_wait) > maxw:
                    waits = list(si.on_wait)
                    head, keep = waits[:-maxw], waits[-maxw:]
                    si.on_wait = keep
                    for i in range(0, len(head), maxw):
                        nop = mybir.InstNoOp(
                            name=f"{inst.name}-ws{i}",
                            engine=inst.engine,
                            ins=[],
                            outs=[],
                            sync_info=mybir.SyncInfo(
                                on_wait=head[i : i + maxw], on_update=[]
                            ),
                        )
                        new_insts.append(nop)
                new_insts.append(inst)
            bb.instructions[:] = new_insts


def _build():
    global _built
    if _built is not None:
        return _built

    nc = bass.Bass()
    dp = nc.declare_dram_parameter
    Xs = dp("Xs", [B, NSH, L], F32, isOutput=False)
    XTs = dp("XTs", [B, 256, NSH], F32, isOutput=False)
    AT = dp("AT", [N, NSH], F32, isOutput=False)
    W1 = dp("W1", [NSH, 48], F32, isOutput=False)
    B1 = dp("B1", [NSH, 16], F32, isOutput=False)
    W2 = dp("W2", [NSH, 48], F32, isOutput=False)
    B2 = dp("B2", [NSH, 1], F32, isOutput=False)
    Wg = dp("Wg", [256, C], F32, isOutput=False)
    L1W = dp("L1W", [256, C], F32, isOutput=False)
    CV = dp("CV", [128, 4 * C], F32, isOutput=False)
    OW = dp("OW", [C, 16], F32, isOutput=False)
    OB = dp("OB", [16, 1], F32, isOutput=False)
    IDN = dp("IDN", [128, 128], F32, isOutput=False)
    out_pool = dp("out_pool", [128, 2 * B], F32, isOutput=True)
    out_cls = dp("out_cls", [16, B], F32, isOutput=True)
    if DEBUG:
        dbg_h1 = dp("dbg_h1", [128, L1], F32, isOutput=True)
        dbg_h = dp("dbg_h", [128, L2], F32, isOutput=True)
        dbg_xw = dp("dbg_xw", [128, C], F32, isOutput=True)
        dbg_lin = dp("dbg_lin", [128, C], F32, isOutput=True)
        dbg_T = dp("dbg_T", [128, C], F32, isOutput=True)
        dbg_tf = dp("dbg_tf", [128, C], F32, isOutput=True)
        dbg_pv = dp("dbg_pv", [128, 2 * B], F32, isOutput=True)
        dbg_rv = dp("dbg_rv", [128, 2 * B], F32, isOutput=True)

    with tile.TileContext(nc) as tc:
        with tc.tile_pool(name="const", bufs=1) as cpool, tc.tile_pool(
            name="keep", bufs=1
        ) as keep, tc.tile_pool(name="dram", bufs=1, space="DRAM") as dram:
            # --- constants ---
            wg0 = cpool.tile([128, C], F32, tag="wg0")
            wg1 = cpool.tile([128, C], F32, tag="wg1")
            nc.sync.dma_start(wg0[:], Wg[0:128, :])
            nc.sync.dma_start(wg1[:], Wg[128:256, :])
            l1w0 = cpool.tile([128, C], F32, tag="l1w0")
            l1w1 = cpool.tile([128, C], F32, tag="l1w1")
            nc.sync.dma_start(l1w0[:], L1W[0:128, :])
            nc.sync.dma_start(l1w1[:], L1W[128:256, :])
            cb = cpool.tile([128, 4 * C], F32, tag="cb")
            nc.sync.dma_start(cb[:], CV[:])
            bgcn = cb[:, 0:C]
            gmsq = cb[:, C : 2 * C]
            gnw = cb[:, 2 * C : 3 * C]
            cbias = cb[:, 3 * C : 4 * C]
            outw = cpool.tile([128, 2, 16], F32, tag="outw")
            nc.sync.dma_start(outw[:], OW.rearrange("(q p) k -> p q k", p=128))
            outb = cpool.tile([16, 1], F32, tag="outb")
            nc.sync.dma_start(outb[:], OB[:])
            ident = cpool.tile([128, 128], F32, tag="ident")
            nc.sync.dma_start(ident[:], IDN[:])
            ones_col = cpool.tile([128, 1], F32, tag="ones")
            nc.gpsimd.memset(ones_col[:], 1.0)
            eps_col = cpool.tile([128, 1], F32, tag="eps")
            nc.gpsimd.memset(eps_col[:], GN_EPS)

            # lin1 output lives across phases; later overwritten in place
            # with the final pre-pool activations.
            lin1_all = keep.tile([128, NT * B, C], F32, tag="lin1all")

            # collective buffers
            cc_in = dram.tile([NSH, B, C], F32, tag="ccin")
            cc_out = dram.tile([N, B, C], F32, tag="ccout")
            cc2_in = dram.tile([128, 2 * B], F32, tag="cc2in")
            cc2_out = dram.tile([128, 2 * B], F32, tag="cc2out")

            # ---------------- Phase A: convs + xw + lin1 per node tile ----
            with tc.tile_pool(name="pa_sb", bufs=2) as pa, tc.tile_pool(
                name="pa_w", bufs=2
            ) as pw, tc.tile_pool(name="pa_diag", bufs=1) as pd, tc.tile_pool(
                name="pa_x", bufs=3
            ) as px, tc.tile_pool(
                name="ps_c1", bufs=3, space="PSUM"
            ) as ps_c1, tc.tile_pool(
                name="ps_c2", bufs=2, space="PSUM"
            ) as ps_c2, tc.tile_pool(
                name="ps_misc", bufs=3, space="PSUM"
            ) as ps_misc:
                for nt in range(NT):
                    nlo = nt * 128
                    w1_sb = pw.tile([128, 48], F32, tag="w1")
                    w2_sb = pw.tile([128, 48], F32, tag="w2")
                    b1_sb = pw.tile([128, 16], F32, tag="b1")
                    b2_sb = pw.tile([128, 1], F32, tag="b2")
                    nc.sync.dma_start(w1_sb[:], W1[nlo : nlo + 128, :])
                    nc.sync.dma_start(w2_sb[:], W2[nlo : nlo + 128, :])
                    nc.sync.dma_start(b1_sb[:], B1[nlo : nlo + 128, :])
                    nc.sync.dma_start(b2_sb[:], B2[nlo : nlo + 128, :])

                    d1 = pd.tile([128, 48, 128], F32, tag="d1")
                    d2 = pd.tile([128, 48, 128], F32, tag="d2")
                    for jk in range(48):
                        nc.vector.tensor_scalar_mul(
                            d1[:, jk, :], ident[:], w1_sb[:, jk : jk + 1]
                        )
                        nc.vector.tensor_scalar_mul(
                            d2[:, jk, :], ident[:], w2_sb[:, jk : jk + 1]
                        )

                    for b in range(B):
                        x_t = px.tile([128, L], F32, tag="x")
                        nc.sync.dma_start(x_t[:], Xs[b, nlo : nlo + 128, :])

                        # conv1 -> h1 [128, 16, 238]
                        h1_t = pa.tile([128, 16, L1], F32, tag="h1")
                        for j in range(16):
                            ps1 = ps_c1.tile([128, L1], F32, tag="c1")
                            for k in range(KW):
                                nc.tensor.matmul(
                                    ps1[:],
                                    d1[:, j * KW + k, :],
                                    x_t[:, k : k + L1],
                                    start=(k == 0),
                                    stop=(k == KW - 1),
                                )
                            nc.scalar.activation(
                                h1_t[:, j, :],
                                ps1[:],
                                AF.Lrelu,
                                bias=b1_sb[:, j : j + 1],
                                alpha=0.01,
                            )

                        # conv2 -> h [128, 236]
                        ps2 = ps_c2.tile([128, L2], F32, tag="c2")
                        for j in range(16):
                            for k in range(KW):
                                nc.tensor.matmul(
                                    ps2[:],
                                    d2[:, j * KW + k, :],
                                    h1_t[:, j, k : k + L2],
                                    start=(j == 0 and k == 0),
                                    stop=(j == 15 and k == KW - 1),
                                )
                        h_t = pa.tile([128, L2], F32, tag="h")
                        nc.scalar.activation(
                            h_t[:],
                            ps2[:],
                            AF.Lrelu,
                            bias=b2_sb[:, 0:1],
                            alpha=0.01,
                        )

                        # transpose h -> hT [t', n] chunks (pad 236->256)
                        pta = ps_misc.tile([128, 256], F32, tag="m")
                        ptb = ps_misc.tile([128, 256], F32, tag="m")
                        nc.tensor.transpose(pta[:, 0:128], h_t[:, 0:128], ident[:])
                        nc.tensor.transpose(
                            ptb[: L2 - 128, 0:128], h_t[:, 128:L2], ident[:]
                        )
                        hT_t = pa.tile([128, 2, 128], F32, tag="hT")
                        nc.vector.tensor_copy(hT_t[:, 0, :], pta[:, 0:128])
                        nc.vector.memset(hT_t[:, 1, :], 0.0)
                        nc.vector.tensor_copy(
                            hT_t[: L2 - 128, 1, :], ptb[: L2 - 128, 0:128]
                        )

                        # xw = h @ Wg  -> [128 nodes, C]
                        psxw = ps_misc.tile([128, 256], F32, tag="m")
                        nc.tensor.matmul(
                            psxw[:, 0:C], hT_t[:, 0, :], wg0[:], start=True, stop=False
                        )
                        nc.tensor.matmul(
                            psxw[:, 0:C], hT_t[:, 1, :], wg1[:], start=False, stop=True
                        )
                        xw_sb = pa.tile([128, C], F32, tag="xw")
                        nc.vector.tensor_copy(xw_sb[:], psxw[:, 0:C])
                        nc.sync.dma_start(cc_in[nlo : nlo + 128, b, :], xw_sb[:])
                        if DEBUG and nt == 0 and b == 0:
                            nc.sync.dma_start(dbg_h1[:], h1_t[:, 0, :])
                            nc.sync.dma_start(dbg_h[:], h_t[:])
                            nc.sync.dma_start(dbg_xw[:], xw_sb[:])

                        # lin1 = X @ L1W -> keep in lin1_all
                        xT_t = px.tile([128, 2, 128], F32, tag="xT")
                        nc.sync.dma_start(
                            xT_t[:],
                            XTs[b, :, nlo : nlo + 128].rearrange(
                                "(q p) n -> p q n", p=128
                            ),
                        )
                        pslin = ps_misc.tile([128, 256], F32, tag="m")
                        nc.tensor.matmul(
                            pslin[:, 0:C], xT_t[:, 0, :], l1w0[:], start=True, stop=False
                        )
                        nc.tensor.matmul(
                            pslin[:, 0:C], xT_t[:, 1, :], l1w1[:], start=False, stop=True
                        )
                        nc.vector.tensor_copy(
                            lin1_all[:, nt * B + b, :], pslin[:, 0:C]
                        )
                        if DEBUG and nt == 0 and b == 0:
                            nc.sync.dma_start(dbg_lin[:], lin1_all[:, 0, :])

            # ---------------- Phase B: AllGather xw ----------------------
            nc.gpsimd.collective_compute(
                "AllGather",
                OP.bypass,
                replica_groups=[list(range(NCORES))],
                ins=[cc_in.opt()],
                outs=[cc_out.opt()],
            )

            # ---------------- Phase C1: aggregate + graphnorm + elu ------
            with tc.tile_pool(name="pc_at", bufs=3) as pat, tc.tile_pool(
                name="pc_rhs", bufs=3
            ) as prhs, tc.tile_pool(name="pc_t", bufs=2) as ptp, tc.tile_pool(
                name="ps_agg", bufs=4, space="PSUM"
            ) as ps_agg:
                aggps = [
                    ps_agg.tile([128, B * C], F32, tag="agg", name=f"agg{i}")
                    for i in range(NT)
                ]
                for st in range(ST):
                    slo = st * 128
                    at_t = pat.tile([128, NSH], F32, tag="at")
                    nc.sync.dma_start(at_t[:], AT[slo : slo + 128, :])
                    xwf_t = prhs.tile([128, B * C], F32, tag="xwf")
                    nc.sync.dma_start(
                        xwf_t[:], cc_out[slo : slo + 128, :, :].rearrange("p b c -> p (b c)")
                    )
                    for nt2 in range(NT):
                        for half in range(2):
                            nc.tensor.matmul(
                                aggps[nt2][:, half * 512 : (half + 1) * 512],
                                at_t[:, nt2 * 128 : (nt2 + 1) * 128],
                                xwf_t[:, half * 512 : (half + 1) * 512],
                                start=(st == 0),
                                stop=(st == ST - 1),
                            )

                for nt2 in range(NT):
                    ps = aggps[nt2]
                    T_t = ptp.tile([128, B, C], F32, tag="T")
                    for b in range(B):
                        nc.vector.tensor_tensor(
                            T_t[:, b, :], ps[:, b * C : (b + 1) * C], bgcn, OP.add
                        )
                    if DEBUG and nt2 == 0:
                        nc.sync.dma_start(dbg_T[:], T_t[:, 0, :])
                    m01 = ptp.tile([128, C], F32, tag="m01")
                    m23 = ptp.tile([128, C], F32, tag="m23")
                    msum = ptp.tile([128, C], F32, tag="msum")
                    nc.vector.tensor_tensor(m01[:], T_t[:, 0, :], T_t[:, 1, :], OP.add)
                    nc.vector.tensor_tensor(m23[:], T_t[:, 2, :], T_t[:, 3, :], OP.add)
                    nc.vector.tensor_tensor(msum[:], m01[:], m23[:], OP.add)
                    m2 = ptp.tile([128, C], F32, tag="m2")
                    nc.vector.tensor_tensor(m2[:], msum[:], gmsq, OP.mult)
                    ctr = ptp.tile([128, B, C], F32, tag="ctr")
                    for b in range(B):
                        nc.vector.tensor_tensor(
                            ctr[:, b, :], T_t[:, b, :], m2[:], OP.subtract
                        )
                    v = ptp.tile([128, C], F32, tag="v")
                    sq = ptp.tile([128, C], F32, tag="sq")
                    nc.vector.tensor_tensor(v[:], ctr[:, 0, :], ctr[:, 0, :], OP.mult)
                    for b in range(1, B):
                        nc.vector.tensor_tensor(
                            sq[:], ctr[:, b, :], ctr[:, b, :], OP.mult
                        )
                        nc.vector.tensor_tensor(v[:], v[:], sq[:], OP.add)
                    # invstd = exp(-0.5 * ln(v/4 + eps))
                    lnv = ptp.tile([128, C], F32, tag="lnv")
                    nc.scalar.activation(
                        lnv[:], v[:], AF.Ln, bias=eps_col[:, 0:1], scale=0.25
                    )
                    inv = ptp.tile([128, C], F32, tag="inv")
                    nc.scalar.activation(inv[:], lnv[:], AF.Exp, scale=-0.5)
                    gwi = ptp.tile([128, C], F32, tag="gwi")
                    nc.vector.tensor_tensor(gwi[:], inv[:], gnw, OP.mult)
                    for b in range(B):
                        slot = lin1_all[:, nt2 * B + b, :]
                        t0 = ptp.tile([128, C], F32, tag="t0")
                        nc.vector.tensor_tensor(t0[:], ctr[:, b, :], gwi[:], OP.mult)
                        nc.vector.tensor_tensor(t0[:], t0[:], slot, OP.add)
                        nc.vector.tensor_tensor(t0[:], t0[:], cbias, OP.add)
                        # elu(x) = max(x,0) + exp(min(x,0)) - 1
                        r = ptp.tile([128, C], F32, tag="r")
                        mn = ptp.tile([128, C], F32, tag="mn")
                        nc.vector.tensor_scalar_max(r[:], t0[:], 0.0)
                        nc.vector.tensor_scalar_min(mn[:], t0[:], 0.0)
                        e = ptp.tile([128, C], F32, tag="e")
                        nc.scalar.activation(e[:], mn[:], AF.Exp)
                        nc.vector.scalar_tensor_tensor(
                            slot, e[:], -1.0, r[:], OP.add, OP.add
                        )
                        if DEBUG and nt2 == 0 and b == 0:
                            nc.sync.dma_start(dbg_tf[:], slot)

            # ---------------- Phase C2: node-mean pool + classifier ------
            with tc.tile_pool(name="pc2", bufs=1) as p2, tc.tile_pool(
                name="ps_p", bufs=1, space="PSUM"
            ) as ps_p:
                pspool = ps_p.tile([128, 2 * B], F32, tag="pool")
                for b in range(B):
                    for cq in range(2):
                        for nt2 in range(NT):
                            nc.tensor.matmul(
                                pspool[:, cq * B + b : cq * B + b + 1],
                                lin1_all[:, nt2 * B + b, cq * 128 : (cq + 1) * 128],
                                ones_col[:],
                                start=(nt2 == 0),
                                stop=(nt2 == NT - 1),
                            )
                poolv = p2.tile([128, 2 * B], F32, tag="poolv")
                nc.vector.tensor_copy(poolv[:], pspool[:])
                nc.sync.dma_start(cc2_in[:], poolv[:])
                if DEBUG:
                    nc.sync.dma_start(dbg_pv[:], poolv[:])
                nc.gpsimd.collective_compute(
                    "AllReduce",
                    OP.add,
                    replica_groups=[list(range(NCORES))],
                    ins=[cc2_in.opt()],
                    outs=[cc2_out.opt()],
                )
                redv = p2.tile([128, 2 * B], F32, tag="redv")
                nc.sync.dma_start(redv[:], cc2_out[:])
                nc.vector.tensor_scalar_mul(redv[:], redv[:], 1.0 / N)
                nc.sync.dma_start(out_pool[:], redv[:])
                if DEBUG:
                    nc.sync.dma_start(dbg_rv[:], redv[:])

                psc = ps_p.tile([16, B], F32, tag="cls")
                nc.tensor.matmul(
                    psc[:], outw[:, 0, :], redv[:, 0:B], start=True, stop=False
                )
                nc.tensor.matmul(
                    psc[:], outw[:, 1, :], redv[:, B : 2 * B], start=False, stop=True
                )
                cls_sb = p2.tile([16, B], F32, tag="clssb")
                nc.vector.tensor_scalar_add(cls_sb[:], psc[:], outb[:, 0:1])
                nc.sync.dma_start(out_cls[:], cls_sb[:])

    _split_sync_waits(nc)
    _built = nc
    return nc


def _host_prep(inputs):
    X = np.asarray(inputs["X"], np.float32)
    edge_index = np.asarray(inputs["edge_index"])
    w1 = np.asarray(inputs["w1"], np.float32)
    b1 = np.asarray(inputs["b1"], np.float32)
    w2 = np.asarray(inputs["w2"], np.float32)
    b2 = np.asarray(inputs["b2"], np.float32)
    W_gcn = np.asarray(inputs["W_gcn"], np.float32)
    b_gcn = np.asarray(inputs["b_gcn"], np.float32)
    gn_weight = np.asarray(inputs["gn_weight"], np.float32)
    gn_bias = np.asarray(inputs["gn_bias"], np.float32)
    gn_mean_scale = np.asarray(inputs["gn_mean_scale"], np.float32)
    lin1_W = np.asarray(inputs["lin1_W"], np.float32)
    lin1_b = np.asarray(inputs["lin1_b"], np.float32)
    out_W = np.asarray(inputs["out_W"], np.float32)
    out_b = np.asarray(inputs["out_b"], np.float32)

    src = edge_index[0].astype(np.int64)
    dst = edge_index[1].astype(np.int64)
    deg = np.bincount(dst, minlength=N).astype(np.float32) + 1.0
    dinv = 1.0 / np.sqrt(deg)
    norm = (dinv[src] * dinv[dst]).astype(np.float32)

    AT_full = np.zeros((N, N), np.float32)
    np.add.at(AT_full, (src, dst), norm)
    AT_full[np.arange(N), np.arange(N)] += dinv * dinv

    Xt = np.ascontiguousarray(X.transpose(0, 2, 1))  # [B, L, N]

    Wg_pad = np.zeros((256, C), np.float32)
    Wg_pad[:L2] = W_gcn
    L1W_pad = np.zeros((256, C), np.float32)
    L1W_pad[:L] = lin1_W
    cv = np.broadcast_to(
        np.concatenate(
            [b_gcn, gn_mean_scale * 0.25, gn_weight, gn_bias + lin1_b]
        ).astype(np.float32)[None, :],
        (128, 4 * C),
    ).copy()
    ident = np.eye(128, dtype=np.float32)

    w1n = w1.reshape(N, 16, KW)
    b1n = b1.reshape(N, 16)
    w2n = w2.reshape(N, 16, KW)

    in_maps = []
    for i in range(NCORES):
        lo, hi = i * NSH, (i + 1) * NSH
        XTs_i = np.zeros((B, 256, NSH), np.float32)
        XTs_i[:, :L, :] = Xt[:, :, lo:hi]
        in_maps.append(
            {
                "Xs": np.ascontiguousarray(X[:, lo:hi, :]),
                "XTs": XTs_i,
                "AT": np.ascontiguousarray(AT_full[:, lo:hi]),
                "W1": np.ascontiguousarray(w1n[lo:hi].reshape(NSH, 48)),
                "B1": np.ascontiguousarray(b1n[lo:hi]),
                "W2": np.ascontiguousarray(w2n[lo:hi].reshape(NSH, 48)),
                "B2": np.ascontiguousarray(b2[lo:hi, None]),
                "Wg": Wg_pad,
                "L1W": L1W_pad,
                "CV": cv,
                "OW": out_W,
                "OB": out_b[:, None].astype(np.float32),
                "IDN": ident,
            }
        )
    return in_maps


def kernel(**inputs):
    nc = _build()
    in_maps = _host_prep(inputs)
    res = run_bass_kernel_spmd(nc, in_maps, list(range(NCORES)))
    op = res.results[0]["out_pool"]  # [128, 2B]  (c_low, cq*B + b)
    oc = res.results[0]["out_cls"]  # [16, B]
    output = np.empty((B, C), np.float32)
    for cq in range(2):
        for b in range(B):
            output[b, cq * 128 : (cq + 1) * 128] = op[:, cq * B + b]
    class_output = np.ascontiguousarray(oc.T)
    return (output, class_output)


# revision 17
# speedup vs baseline: 2.2812x; 2.2812x over previous
"""Trainium2 Bass kernel for nn_Body_GNN (gnn_message_passing).

Sharding: nodes are split 512/core across 8 NeuronCores. Per core:
  - grouped conv1/conv2 (per-node weights) run on the tensor engine as
    diagonal-matrix matmuls with tap accumulation in PSUM,
  - xw = h @ W_gcn and lin1 = X @ lin1_W as standard matmuls
    (X pre-transposed on host, h transposed on the PE),
  - xw shards exchanged with AllGather,
  - GCN aggregate as dense A_shard @ xw_full (A built on host from
    edge_index: symmetric norms + self-loop diagonal),
  - GraphNorm (stats over batch) + elu on vector/scalar engines,
  - node-mean pool via ones-vector matmul + AllReduce.
"""
import sys

sys.path.insert(0, "/opt/trn_rl_repo")

import numpy as np
import concourse.bass as bass
import concourse.mybir as mybir
import concourse.tile as tile
from concourse.vector_clock import ScopedClock
from concourse.bass_utils import run_bass_kernel_spmd

# ---------------------------------------------------------------------------
# Workaround: CoreV3 codegen in this toolchain rejects >1 sync-wait on the
# TileContext tail Drain; split the waits across sync-engine NOPs instead.
_MAXW = 1


def _patched_drain_and_barrier(self, tick_clock, wait_clock):
    nc = self.nc
    probe = nc.sync.nop(nofuse=True)
    wait_clock.add_sem_waits(probe.ins, ScopedClock({None: tick_clock.global_clock}))
    si = probe.ins.sync_info
    waits = list(si.on_wait) if si and si.on_wait else []
    if len(waits) > _MAXW:
        si.on_wait = waits[:_MAXW]
        rest = waits[_MAXW:]
        for i in range(0, len(rest), _MAXW):
            n2 = nc.sync.nop(nofuse=True)
            if n2.ins.sync_info is None:
                n2.ins.sync_info = mybir.SyncInfo(
                    on_wait=rest[i : i + _MAXW], on_update=[]
                )
            else:
                n2.ins.sync_info.on_wait = rest[i : i + _MAXW]
    nc.sync.drain()
    nc.all_engine_barrier()
    assert self.sems is not None
    popped = nc._tile_sem_poison_stack.pop()
    assert popped is self._sem_poison
    nc.clear_and_free_semaphores(list(self.sems.allocated().values()))
    nc.all_engine_barrier()


tile.TileContext._drain_and_barrier = _patched_drain_and_barrier

# ---------------------------------------------------------------------------
B, N, L, KW, C = 4, 4096, 240, 3, 256
L1 = L - KW + 1  # 238, conv1 output length
L2 = L1 - KW + 1  # 236, conv2 output length
NCORES = 8
NSH = N // NCORES  # 512 nodes per core
NT = NSH // 128  # 4 node tiles per core
ST = N // 128  # 32 source k-tiles for the aggregate
GN_EPS = 1e-5

AF = mybir.ActivationFunctionType
OP = mybir.AluOpType
F32 = mybir.dt.float32
F32R = mybir.dt.float32r

_built = None
DEBUG = False

# This walrus build rejects instructions carrying more than _MAX_ISA_WAITS
# sync waits. Hoist the excess onto same-engine NoOps placed just before.
_MAX_ISA_WAITS = 1


def _split_sync_waits(nc, maxw=_MAX_ISA_WAITS):
    for f in nc.m.functions:
        for bb in f.blocks:
            new_insts = []
            for inst in bb.instructions:
                si = inst.sync_info
                if si is not None and si.on_wait and len(si.on_wait) > maxw:
                    waits = list(si.on_wait)
                    head, keep = waits[:-maxw], waits[-maxw:]
                    si.on_wait = keep
                    for i in range(0, len(head), maxw):
                        nop = mybir.InstNoOp(
                            name=f"{inst.name}-ws{i}",
                            engine=inst.engine,
                            ins=[],
                            outs=[],
                            sync_info=mybir.SyncInfo(
                                on_wait=head[i : i + maxw], on_update=[]
                            ),
                        )
                        new_insts.append(nop)
                new_insts.append(inst)
            bb.instructions[:] = new_insts


def _build():
    global _built
    if _built is not None:
        return _built

    nc = bass.Bass()
    dp = nc.declare_dram_parameter
    Xs = dp("Xs", [B, NSH, L], F32R, isOutput=False)
    XTs = dp("XTs", [B, 256, NSH], F32R, isOutput=False)
    AT = dp("AT", [N, NSH], F32R, isOutput=False)
    W1 = dp("W1", [NSH, 48], F32, isOutput=False)
    B1 = dp("B1", [NSH, 16], F32, isOutput=False)
    W2 = dp("W2", [NSH, 48], F32, isOutput=False)
    B2 = dp("B2", [NSH, 1], F32, isOutput=False)
    Wg = dp("Wg", [256, C], F32R, isOutput=False)
    L1W = dp("L1W", [256, C], F32R, isOutput=False)
    CV = dp("CV", [128, 4 * C], F32, isOutput=False)
    OW = dp("OW", [C, 16], F32, isOutput=False)
    OB = dp("OB", [16, 1], F32, isOutput=False)
    IDN = dp("IDN", [128, 128], F32R, isOutput=False)
    ZZ = dp("ZZ", [128, 128], F32R, isOutput=False)
    out_pool = dp("out_pool", [128, 2 * B], F32, isOutput=True)
    out_cls = dp("out_cls", [16, B], F32, isOutput=True)
    if DEBUG:
        dbg_h1 = dp("dbg_h1", [128, L1], F32, isOutput=True)
        dbg_h = dp("dbg_h", [128, L2], F32, isOutput=True)
        dbg_xw = dp("dbg_xw", [128, C], F32, isOutput=True)
        dbg_lin = dp("dbg_lin", [128, C], F32, isOutput=True)
        dbg_T = dp("dbg_T", [128, C], F32, isOutput=True)
        dbg_tf = dp("dbg_tf", [128, C], F32, isOutput=True)
        dbg_pv = dp("dbg_pv", [128, 2 * B], F32, isOutput=True)
        dbg_rv = dp("dbg_rv", [128, 2 * B], F32, isOutput=True)

    with tile.TileContext(nc) as tc:
        with tc.tile_pool(name="const", bufs=1) as cpool, tc.tile_pool(
            name="keep", bufs=1
        ) as keep, tc.tile_pool(name="dram", bufs=1, space="DRAM") as dram:
            # --- constants ---
            wg0 = cpool.tile([128, C], F32R, tag="wg0")
            wg1 = cpool.tile([128, C], F32R, tag="wg1")
            nc.sync.dma_start(wg0[:], Wg[0:128, :])
            nc.sync.dma_start(wg1[:], Wg[128:256, :])
            l1w0 = cpool.tile([128, C], F32R, tag="l1w0")
            l1w1 = cpool.tile([128, C], F32R, tag="l1w1")
            nc.sync.dma_start(l1w0[:], L1W[0:128, :])
            nc.sync.dma_start(l1w1[:], L1W[128:256, :])
            cb = cpool.tile([128, 4 * C], F32, tag="cb")
            nc.sync.dma_start(cb[:], CV[:])
            bgcn = cb[:, 0:C]
            gmsq = cb[:, C : 2 * C]
            gnw = cb[:, 2 * C : 3 * C]
            cbias = cb[:, 3 * C : 4 * C]
            outw = cpool.tile([128, 2, 16], F32, tag="outw")
            nc.sync.dma_start(outw[:], OW.rearrange("(q p) k -> p q k", p=128))
            outb = cpool.tile([16, 1], F32, tag="outb")
            nc.sync.dma_start(outb[:], OB[:])
            ident = cpool.tile([128, 128], F32R, tag="ident")
            nc.sync.dma_start(ident[:], IDN[:])
            ones_col = cpool.tile([128, 1], F32, tag="ones")
            nc.gpsimd.memset(ones_col[:], 1.0)
            eps_col = cpool.tile([128, 1], F32, tag="eps")
            nc.gpsimd.memset(eps_col[:], GN_EPS)

            # lin1 output lives across phases; later overwritten in place
            # with the final pre-pool activations.
            lin1_all = keep.tile([128, NT * B, C], F32, tag="lin1all")

            # collective buffers
            cc_in = dram.tile([NSH, B, C], F32R, tag="ccin")
            cc_out = dram.tile([N, B, C], F32R, tag="ccout")
            cc2_in = dram.tile([128, 2 * B], F32, tag="cc2in")
            cc2_out = dram.tile([128, 2 * B], F32, tag="cc2out")

            # ---------------- Phase A: convs + xw + lin1 per node tile ----
            with tc.tile_pool(name="pa_sb", bufs=2) as pa, tc.tile_pool(
                name="pa_w", bufs=2
            ) as pw, tc.tile_pool(name="pa_diag", bufs=1) as pd, tc.tile_pool(
                name="pa_x", bufs=3
            ) as px, tc.tile_pool(
                name="ps_c1", bufs=2, space="PSUM"
            ) as ps_c1, tc.tile_pool(
                name="ps_c2", bufs=2, space="PSUM"
            ) as ps_c2, tc.tile_pool(
                name="ps_misc", bufs=2, space="PSUM"
            ) as ps_misc, tc.tile_pool(
                name="ps_tr", bufs=2, space="PSUM"
            ) as ps_tr:
                for nt in range(NT):
                    nlo = nt * 128
                    w1_sb = pw.tile([128, 48], F32, tag="w1")
                    w2_sb = pw.tile([128, 48], F32, tag="w2")
                    b1_sb = pw.tile([128, 16], F32, tag="b1")
                    b2_sb = pw.tile([128, 1], F32, tag="b2")
                    nc.sync.dma_start(w1_sb[:], W1[nlo : nlo + 128, :])
                    nc.sync.dma_start(w2_sb[:], W2[nlo : nlo + 128, :])
                    nc.sync.dma_start(b1_sb[:], B1[nlo : nlo + 128, :])
                    nc.sync.dma_start(b2_sb[:], B2[nlo : nlo + 128, :])

                    d1 = pd.tile([128, 48, 128], F32R, tag="d1")
                    d2 = pd.tile([128, 48, 128], F32R, tag="d2")
                    for jk in range(48):
                        nc.vector.tensor_scalar_mul(
                            d1[:, jk, :], ident[:], w1_sb[:, jk : jk + 1]
                        )
                        nc.vector.tensor_scalar_mul(
                            d2[:, jk, :], ident[:], w2_sb[:, jk : jk + 1]
                        )

                    for bp in range(B // 2):
                        b0 = bp * 2
                        # two batches side by side: N=476 >= 256 keeps the
                        # f32r matmuls at 1 cycle/row
                        x_t = px.tile([128, 2, L], F32R, tag="x")
                        nc.sync.dma_start(
                            x_t[:],
                            Xs[b0 : b0 + 2, nlo : nlo + 128, :].rearrange(
                                "b p t -> p b t"
                            ),
                        )

                        # conv1 -> h1 [128, 2, 16, 238]
                        h1_t = pa.tile([128, 2, 16, L1], F32R, tag="h1")
                        for j in range(16):
                            ps1 = ps_c1.tile([128, 2, L1], F32, tag="c1")
                            for k in range(KW):
                                nc.tensor.matmul(
                                    ps1[:],
                                    d1[:, j * KW + k, :],
                                    x_t[:, :, k : k + L1],
                                    start=(k == 0),
                                    stop=(k == KW - 1),
                                )
                            nc.scalar.activation(
                                h1_t[:, :, j, :],
                                ps1[:],
                                AF.Lrelu,
                                bias=b1_sb[:, j : j + 1],
                                alpha=0.01,
                            )

                        # conv2 -> h [128, 2, 236]
                        ps2 = ps_c2.tile([128, 2, L2], F32, tag="c2")
                        for j in range(16):
                            for k in range(KW):
                                nc.tensor.matmul(
                                    ps2[:],
                                    d2[:, j * KW + k, :],
                                    h1_t[:, :, j, k : k + L2],
                                    start=(j == 0 and k == 0),
                                    stop=(j == 15 and k == KW - 1),
                                )
                        h_t = pa.tile([128, 2, L2], F32R, tag="h")
                        nc.scalar.activation(
                            h_t[:],
                            ps2[:],
                            AF.Lrelu,
                            bias=b2_sb[:, 0:1],
                            alpha=0.01,
                        )

                        for bi in range(2):
                            b = b0 + bi
                            # transpose h -> hT [t', n] chunks (pad 236->256)
                            pta = ps_tr.tile([128, 256], F32R, tag="mt")
                            ptb = ps_tr.tile([128, 256], F32R, tag="mt")
                            nc.tensor.transpose(
                                pta[:, 0:128], h_t[:, bi, 0:128], ident[:]
                            )
                            nc.tensor.transpose(
                                ptb[: L2 - 128, 0:128], h_t[:, bi, 128:L2], ident[:]
                            )
                            hT_t = pa.tile([128, 2, 128], F32R, tag="hT")
                            nc.vector.tensor_copy(hT_t[:, 0, :], pta[:, 0:128])
                            nc.sync.dma_start(hT_t[:, 1, :], ZZ[:])
                            nc.vector.tensor_copy(
                                hT_t[: L2 - 128, 1, :], ptb[: L2 - 128, 0:128]
                            )

                            # xw = h @ Wg  -> [128 nodes, C]
                            psxw = ps_misc.tile([128, 256], F32, tag="m")
                            nc.tensor.matmul(
                                psxw[:, 0:C],
                                hT_t[:, 0, :],
                                wg0[:],
                                start=True,
                                stop=False,
                            )
                            nc.tensor.matmul(
                                psxw[:, 0:C],
                                hT_t[:, 1, :],
                                wg1[:],
                                start=False,
                                stop=True,
                            )
                            xw_sb = pa.tile([128, C], F32R, tag="xw")
                            nc.vector.tensor_copy(xw_sb[:], psxw[:, 0:C])
                            nc.sync.dma_start(cc_in[nlo : nlo + 128, b, :], xw_sb[:])
                            if DEBUG and nt == 0 and b == 0:
                                nc.sync.dma_start(dbg_h1[:], h1_t[:, 0, 0, :])
                                nc.sync.dma_start(dbg_h[:], h_t[:, 0, :])
                                nc.sync.dma_start(dbg_xw[:], xw_sb[:])

                            # lin1 = X @ L1W -> keep in lin1_all
                            xT_t = px.tile([128, 2, 128], F32R, tag="xT")
                            nc.sync.dma_start(
                                xT_t[:],
                                XTs[b, :, nlo : nlo + 128].rearrange(
                                    "(q p) n -> p q n", p=128
                                ),
                            )
                            pslin = ps_misc.tile([128, 256], F32, tag="m")
                            nc.tensor.matmul(
                                pslin[:, 0:C],
                                xT_t[:, 0, :],
                                l1w0[:],
                                start=True,
                                stop=False,
                            )
                            nc.tensor.matmul(
                                pslin[:, 0:C],
                                xT_t[:, 1, :],
                                l1w1[:],
                                start=False,
                                stop=True,
                            )
                            nc.vector.tensor_copy(
                                lin1_all[:, nt * B + b, :], pslin[:, 0:C]
                            )
                            if DEBUG and nt == 0 and b == 0:
                                nc.sync.dma_start(dbg_lin[:], lin1_all[:, 0, :])

            # ---------------- Phase B: AllGather xw ----------------------
            nc.gpsimd.collective_compute(
                "AllGather",
                OP.bypass,
                replica_groups=[list(range(NCORES))],
                ins=[cc_in.opt()],
                outs=[cc_out.opt()],
            )

            # ---------------- Phase C1: aggregate + graphnorm + elu ------
            with tc.tile_pool(name="pc_at", bufs=3) as pat, tc.tile_pool(
                name="pc_rhs", bufs=3
            ) as prhs, tc.tile_pool(name="pc_t", bufs=2) as ptp, tc.tile_pool(
                name="ps_agg", bufs=4, space="PSUM"
            ) as ps_agg:
                aggps = [
                    ps_agg.tile([128, B * C], F32, tag="agg", name=f"agg{i}")
                    for i in range(NT)
                ]
                for st in range(ST):
                    slo = st * 128
                    at_t = pat.tile([128, NSH], F32R, tag="at")
                    nc.sync.dma_start(at_t[:], AT[slo : slo + 128, :])
                    xwf_t = prhs.tile([128, B * C], F32R, tag="xwf")
                    nc.sync.dma_start(
                        xwf_t[:], cc_out[slo : slo + 128, :, :].rearrange("p b c -> p (b c)")
                    )
                    for nt2 in range(NT):
                        for half in range(2):
                            nc.tensor.matmul(
                                aggps[nt2][:, half * 512 : (half + 1) * 512],
                                at_t[:, nt2 * 128 : (nt2 + 1) * 128],
                                xwf_t[:, half * 512 : (half + 1) * 512],
                                start=(st == 0),
                                stop=(st == ST - 1),
                            )

                for nt2 in range(NT):
                    ps = aggps[nt2]
                    T_t = ptp.tile([128, B, C], F32, tag="T")
                    for b in range(B):
                        nc.vector.tensor_tensor(
                            T_t[:, b, :], ps[:, b * C : (b + 1) * C], bgcn, OP.add
                        )
                    if DEBUG and nt2 == 0:
                        nc.sync.dma_start(dbg_T[:], T_t[:, 0, :])
                    m01 = ptp.tile([128, C], F32, tag="m01")
                    m23 = ptp.tile([128, C], F32, tag="m23")
                    msum = ptp.tile([128, C], F32, tag="msum")
                    nc.vector.tensor_tensor(m01[:], T_t[:, 0, :], T_t[:, 1, :], OP.add)
                    nc.vector.tensor_tensor(m23[:], T_t[:, 2, :], T_t[:, 3, :], OP.add)
                    nc.vector.tensor_tensor(msum[:], m01[:], m23[:], OP.add)
                    m2 = ptp.tile([128, C], F32, tag="m2")
                    nc.vector.tensor_tensor(m2[:], msum[:], gmsq, OP.mult)
                    ctr = ptp.tile([128, B, C], F32, tag="ctr")
                    for b in range(B):
                        nc.vector.tensor_tensor(
                            ctr[:, b, :], T_t[:, b, :], m2[:], OP.subtract
                        )
                    v = ptp.tile([128, C], F32, tag="v")
                    sq = ptp.tile([128, C], F32, tag="sq")
                    nc.vector.tensor_tensor(v[:], ctr[:, 0, :], ctr[:, 0, :], OP.mult)
                    for b in range(1, B):
                        nc.vector.tensor_tensor(
                            sq[:], ctr[:, b, :], ctr[:, b, :], OP.mult
                        )
                        nc.vector.tensor_tensor(v[:], v[:], sq[:], OP.add)
                    # invstd = exp(-0.5 * ln(v/4 + eps))
                    lnv = ptp.tile([128, C], F32, tag="lnv")
                    nc.scalar.activation(
                        lnv[:], v[:], AF.Ln, bias=eps_col[:, 0:1], scale=0.25
                    )
                    inv = ptp.tile([128, C], F32, tag="inv")
                    nc.scalar.activation(inv[:], lnv[:], AF.Exp, scale=-0.5)
                    gwi = ptp.tile([128, C], F32, tag="gwi")
                    nc.vector.tensor_tensor(gwi[:], inv[:], gnw, OP.mult)
                    for b in range(B):
                        slot = lin1_all[:, nt2 * B + b, :]
                        t0 = ptp.tile([128, C], F32, tag="t0")
                        nc.vector.tensor_tensor(t0[:], ctr[:, b, :], gwi[:], OP.mult)
                        nc.vector.tensor_tensor(t0[:], t0[:], slot, OP.add)
                        nc.vector.tensor_tensor(t0[:], t0[:], cbias, OP.add)
                        # elu(x) = max(x,0) + exp(min(x,0)) - 1
                        r = ptp.tile([128, C], F32, tag="r")
                        mn = ptp.tile([128, C], F32, tag="mn")
                        nc.vector.tensor_scalar_max(r[:], t0[:], 0.0)
                        nc.vector.tensor_scalar_min(mn[:], t0[:], 0.0)
                        e = ptp.tile([128, C], F32, tag="e")
                        nc.scalar.activation(e[:], mn[:], AF.Exp)
                        nc.vector.scalar_tensor_tensor(
                            slot, e[:], -1.0, r[:], OP.add, OP.add
                        )
                        if DEBUG and nt2 == 0 and b == 0:
                            nc.sync.dma_start(dbg_tf[:], slot)

            # ---------------- Phase C2: node-mean pool + classifier ------
            with tc.tile_pool(name="pc2", bufs=1) as p2, tc.tile_pool(
                name="ps_p", bufs=1, space="PSUM"
            ) as ps_p:
                pspool = ps_p.tile([128, 2 * B], F32, tag="pool")
                for b in range(B):
                    for cq in range(2):
                        for nt2 in range(NT):
                            nc.tensor.matmul(
                                pspool[:, cq * B + b : cq * B + b + 1],
                                lin1_all[:, nt2 * B + b, cq * 128 : (cq + 1) * 128],
                                ones_col[:],
                                start=(nt2 == 0),
                                stop=(nt2 == NT - 1),
                            )
                poolv = p2.tile([128, 2 * B], F32, tag="poolv")
                nc.vector.tensor_copy(poolv[:], pspool[:])
                nc.sync.dma_start(cc2_in[:], poolv[:])
                if DEBUG:
                    nc.sync.dma_start(dbg_pv[:], poolv[:])
                nc.gpsimd.collective_compute(
                    "AllReduce",
                    OP.add,
                    replica_groups=[list(range(NCORES))],
                    ins=[cc2_in.opt()],
                    outs=[cc2_out.opt()],
                )
                redv = p2.tile([128, 2 * B], F32, tag="redv")
                nc.sync.dma_start(redv[:], cc2_out[:])
                nc.vector.tensor_scalar_mul(redv[:], redv[:], 1.0 / N)
                nc.sync.dma_start(out_pool[:], redv[:])
                if DEBUG:
                    nc.sync.dma_start(dbg_rv[:], redv[:])

                psc = ps_p.tile([16, B], F32, tag="cls")
                nc.tensor.matmul(
                    psc[:], outw[:, 0, :], redv[:, 0:B], start=True, stop=False
                )
                nc.tensor.matmul(
                    psc[:], outw[:, 1, :], redv[:, B : 2 * B], start=False, stop=True
                )
                cls_sb = p2.tile([16, B], F32, tag="clssb")
                nc.vector.tensor_scalar_add(cls_sb[:], psc[:], outb[:, 0:1])
                nc.sync.dma_start(out_cls[:], cls_sb[:])

    _split_sync_waits(nc)
    _built = nc
    return nc


def _host_prep(inputs):
    X = np.asarray(inputs["X"], np.float32)
    edge_index = np.asarray(inputs["edge_index"])
    w1 = np.asarray(inputs["w1"], np.float32)
    b1 = np.asarray(inputs["b1"], np.float32)
    w2 = np.asarray(inputs["w2"], np.float32)
    b2 = np.asarray(inputs["b2"], np.float32)
    W_gcn = np.asarray(inputs["W_gcn"], np.float32)
    b_gcn = np.asarray(inputs["b_gcn"], np.float32)
    gn_weight = np.asarray(inputs["gn_weight"], np.float32)
    gn_bias = np.asarray(inputs["gn_bias"], np.float32)
    gn_mean_scale = np.asarray(inputs["gn_mean_scale"], np.float32)
    lin1_W = np.asarray(inputs["lin1_W"], np.float32)
    lin1_b = np.asarray(inputs["lin1_b"], np.float32)
    out_W = np.asarray(inputs["out_W"], np.float32)
    out_b = np.asarray(inputs["out_b"], np.float32)

    src = edge_index[0].astype(np.int64)
    dst = edge_index[1].astype(np.int64)
    deg = np.bincount(dst, minlength=N).astype(np.float32) + 1.0
    dinv = 1.0 / np.sqrt(deg)
    norm = (dinv[src] * dinv[dst]).astype(np.float32)

    AT_full = np.zeros((N, N), np.float32)
    np.add.at(AT_full, (src, dst), norm)
    AT_full[np.arange(N), np.arange(N)] += dinv * dinv

    Xt = np.ascontiguousarray(X.transpose(0, 2, 1))  # [B, L, N]

    Wg_pad = np.zeros((256, C), np.float32)
    Wg_pad[:L2] = W_gcn
    L1W_pad = np.zeros((256, C), np.float32)
    L1W_pad[:L] = lin1_W
    cv = np.broadcast_to(
        np.concatenate(
            [b_gcn, gn_mean_scale * 0.25, gn_weight, gn_bias + lin1_b]
        ).astype(np.float32)[None, :],
        (128, 4 * C),
    ).copy()
    ident = np.eye(128, dtype=np.float32)

    w1n = w1.reshape(N, 16, KW)
    b1n = b1.reshape(N, 16)
    w2n = w2.reshape(N, 16, KW)

    in_maps = []
    for i in range(NCORES):
        lo, hi = i * NSH, (i + 1) * NSH
        XTs_i = np.zeros((B, 256, NSH), np.float32)
        XTs_i[:, :L, :] = Xt[:, :, lo:hi]
        in_maps.append(
            {
                "Xs": np.ascontiguousarray(X[:, lo:hi, :]),
                "XTs": XTs_i,
                "AT": np.ascontiguousarray(AT_full[:, lo:hi]),
                "W1": np.ascontiguousarray(w1n[lo:hi].reshape(NSH, 48)),
                "B1": np.ascontiguousarray(b1n[lo:hi]),
                "W2": np.ascontiguousarray(w2n[lo:hi].reshape(NSH, 48)),
                "B2": np.ascontiguousarray(b2[lo:hi, None]),
                "Wg": Wg_pad,
                "L1W": L1W_pad,
                "CV": cv,
                "OW": out_W,
                "OB": out_b[:, None].astype(np.float32),
                "IDN": ident,
                "ZZ": np.zeros((128, 128), np.float32),
            }
        )
    return in_maps


def kernel(**inputs):
    nc = _build()
    in_maps = _host_prep(inputs)
    res = run_bass_kernel_spmd(nc, in_maps, list(range(NCORES)))
    op = res.results[0]["out_pool"]  # [128, 2B]  (c_low, cq*B + b)
    oc = res.results[0]["out_cls"]  # [16, B]
    output = np.empty((B, C), np.float32)
    for cq in range(2):
        for b in range(B):
            output[b, cq * 128 : (cq + 1) * 128] = op[:, cq * B + b]
    class_output = np.ascontiguousarray(oc.T)
    return (output, class_output)


# revision 19
# speedup vs baseline: 2.8288x; 1.2401x over previous
"""Trainium2 Bass kernel for nn_Body_GNN (gnn_message_passing).

Sharding: nodes are split 512/core across 8 NeuronCores. Per core:
  - grouped conv1/conv2 (per-node weights) run on the tensor engine as
    diagonal-matrix matmuls with tap accumulation in PSUM,
  - xw = h @ W_gcn and lin1 = X @ lin1_W as standard matmuls
    (X pre-transposed on host, h transposed on the PE),
  - xw shards exchanged with AllGather,
  - GCN aggregate as dense A_shard @ xw_full (A built on host from
    edge_index: symmetric norms + self-loop diagonal),
  - GraphNorm (stats over batch) + elu on vector/scalar engines,
  - node-mean pool via ones-vector matmul + AllReduce.
"""
import sys

sys.path.insert(0, "/opt/trn_rl_repo")

import numpy as np
import concourse.bass as bass
import concourse.mybir as mybir
import concourse.tile as tile
from concourse.vector_clock import ScopedClock
from concourse.bass_utils import run_bass_kernel_spmd

# ---------------------------------------------------------------------------
# Workaround: CoreV3 codegen in this toolchain rejects >1 sync-wait on the
# TileContext tail Drain; split the waits across sync-engine NOPs instead.
_MAXW = 1


def _patched_drain_and_barrier(self, tick_clock, wait_clock):
    nc = self.nc
    probe = nc.sync.nop(nofuse=True)
    wait_clock.add_sem_waits(probe.ins, ScopedClock({None: tick_clock.global_clock}))
    si = probe.ins.sync_info
    waits = list(si.on_wait) if si and si.on_wait else []
    if len(waits) > _MAXW:
        si.on_wait = waits[:_MAXW]
        rest = waits[_MAXW:]
        for i in range(0, len(rest), _MAXW):
            n2 = nc.sync.nop(nofuse=True)
            if n2.ins.sync_info is None:
                n2.ins.sync_info = mybir.SyncInfo(
                    on_wait=rest[i : i + _MAXW], on_update=[]
                )
            else:
                n2.ins.sync_info.on_wait = rest[i : i + _MAXW]
    nc.sync.drain()
    nc.all_engine_barrier()
    assert self.sems is not None
    popped = nc._tile_sem_poison_stack.pop()
    assert popped is self._sem_poison
    nc.clear_and_free_semaphores(list(self.sems.allocated().values()))
    nc.all_engine_barrier()


tile.TileContext._drain_and_barrier = _patched_drain_and_barrier

# ---------------------------------------------------------------------------
B, N, L, KW, C = 4, 4096, 240, 3, 256
L1 = L - KW + 1  # 238, conv1 output length
L2 = L1 - KW + 1  # 236, conv2 output length
NCORES = 8
NSH = N // NCORES  # 512 nodes per core
NT = NSH // 128  # 4 node tiles per core
ST = N // 128  # 32 source k-tiles for the aggregate
GN_EPS = 1e-5

AF = mybir.ActivationFunctionType
OP = mybir.AluOpType
F32 = mybir.dt.float32
F32R = mybir.dt.float32r

_built = None
DEBUG = False

# This walrus build rejects instructions carrying more than _MAX_ISA_WAITS
# sync waits. Hoist the excess onto same-engine NoOps placed just before.
_MAX_ISA_WAITS = 1


def _split_sync_waits(nc, maxw=_MAX_ISA_WAITS):
    for f in nc.m.functions:
        for bb in f.blocks:
            new_insts = []
            for inst in bb.instructions:
                si = inst.sync_info
                if si is not None and si.on_wait and len(si.on_wait) > maxw:
                    waits = list(si.on_wait)
                    head, keep = waits[:-maxw], waits[-maxw:]
                    si.on_wait = keep
                    for i in range(0, len(head), maxw):
                        nop = mybir.InstNoOp(
                            name=f"{inst.name}-ws{i}",
                            engine=inst.engine,
                            ins=[],
                            outs=[],
                            sync_info=mybir.SyncInfo(
                                on_wait=head[i : i + maxw], on_update=[]
                            ),
                        )
                        new_insts.append(nop)
                new_insts.append(inst)
            bb.instructions[:] = new_insts


def _build():
    global _built
    if _built is not None:
        return _built

    nc = bass.Bass()
    dp = nc.declare_dram_parameter
    Xs = dp("Xs", [B, NSH, L], F32R, isOutput=False)
    XTs = dp("XTs", [B, 256, NSH], F32R, isOutput=False)
    AT = dp("AT", [N, NSH], F32R, isOutput=False)
    W1 = dp("W1", [NSH, 48], F32, isOutput=False)
    B1 = dp("B1", [NSH, 16], F32, isOutput=False)
    W2 = dp("W2", [NSH, 48], F32, isOutput=False)
    B2 = dp("B2", [NSH, 1], F32, isOutput=False)
    Wg = dp("Wg", [256, C], F32R, isOutput=False)
    L1W = dp("L1W", [256, C], F32R, isOutput=False)
    CV = dp("CV", [128, 4 * C], F32, isOutput=False)
    OW = dp("OW", [C, 16], F32, isOutput=False)
    OB = dp("OB", [16, 1], F32, isOutput=False)
    IDN = dp("IDN", [128, 128], F32R, isOutput=False)
    ZZ = dp("ZZ", [128, 128], F32R, isOutput=False)
    out_pool = dp("out_pool", [128, 2 * B], F32, isOutput=True)
    out_cls = dp("out_cls", [16, B], F32, isOutput=True)
    if DEBUG:
        dbg_h1 = dp("dbg_h1", [128, L1], F32, isOutput=True)
        dbg_h = dp("dbg_h", [128, L2], F32, isOutput=True)
        dbg_xw = dp("dbg_xw", [128, C], F32, isOutput=True)
        dbg_lin = dp("dbg_lin", [128, C], F32, isOutput=True)
        dbg_T = dp("dbg_T", [128, C], F32, isOutput=True)
        dbg_tf = dp("dbg_tf", [128, C], F32, isOutput=True)
        dbg_pv = dp("dbg_pv", [128, 2 * B], F32, isOutput=True)
        dbg_rv = dp("dbg_rv", [128, 2 * B], F32, isOutput=True)

    with tile.TileContext(nc) as tc:
        with tc.tile_pool(name="const", bufs=1) as cpool, tc.tile_pool(
            name="keep", bufs=1
        ) as keep, tc.tile_pool(name="dram", bufs=1, space="DRAM") as dram:
            # --- constants ---
            wg0 = cpool.tile([128, C], F32R, tag="wg0")
            wg1 = cpool.tile([128, C], F32R, tag="wg1")
            nc.sync.dma_start(wg0[:], Wg[0:128, :])
            nc.sync.dma_start(wg1[:], Wg[128:256, :])
            l1w0 = cpool.tile([128, C], F32R, tag="l1w0")
            l1w1 = cpool.tile([128, C], F32R, tag="l1w1")
            nc.sync.dma_start(l1w0[:], L1W[0:128, :])
            nc.sync.dma_start(l1w1[:], L1W[128:256, :])
            cb = cpool.tile([128, 4 * C], F32, tag="cb")
            nc.sync.dma_start(cb[:], CV[:])
            bgcn = cb[:, 0:C]
            gmsq = cb[:, C : 2 * C]
            gnw = cb[:, 2 * C : 3 * C]
            cbias = cb[:, 3 * C : 4 * C]
            outw = cpool.tile([128, 2, 16], F32, tag="outw")
            nc.sync.dma_start(outw[:], OW.rearrange("(q p) k -> p q k", p=128))
            outb = cpool.tile([16, 1], F32, tag="outb")
            nc.sync.dma_start(outb[:], OB[:])
            ident = cpool.tile([128, 128], F32R, tag="ident")
            nc.sync.dma_start(ident[:], IDN[:])
            ones_col = cpool.tile([128, 1], F32, tag="ones")
            nc.gpsimd.memset(ones_col[:], 1.0)
            eps_col = cpool.tile([128, 1], F32, tag="eps")
            nc.gpsimd.memset(eps_col[:], GN_EPS)

            # lin1 output lives across phases; later overwritten in place
            # with the final pre-pool activations.
            lin1_all = keep.tile([128, NT * B, C], F32, tag="lin1all")

            # collective buffers
            cc_in = dram.tile([NSH, B, C], F32R, tag="ccin")
            cc_outs = [
                dram.tile([NCORES * 128, B, C], F32R, tag="ccout", name=f"ccout{i}")
                for i in range(NT)
            ]
            cc2_in = dram.tile([128, 2 * B], F32, tag="cc2in")
            cc2_out = dram.tile([128, 2 * B], F32, tag="cc2out")

            # ---------------- Phase A: convs + xw + lin1 per node tile ----
            with tc.tile_pool(name="pa_sb", bufs=2) as pa, tc.tile_pool(
                name="pa_w", bufs=2
            ) as pw, tc.tile_pool(name="pa_diag", bufs=1) as pd, tc.tile_pool(
                name="pa_x", bufs=3
            ) as px, tc.tile_pool(
                name="ps_c1", bufs=2, space="PSUM"
            ) as ps_c1, tc.tile_pool(
                name="ps_c2", bufs=2, space="PSUM"
            ) as ps_c2, tc.tile_pool(
                name="ps_misc", bufs=2, space="PSUM"
            ) as ps_misc, tc.tile_pool(
                name="ps_tr", bufs=2, space="PSUM"
            ) as ps_tr:
                for nt in range(NT):
                    nlo = nt * 128
                    w1_sb = pw.tile([128, 48], F32, tag="w1")
                    w2_sb = pw.tile([128, 48], F32, tag="w2")
                    b1_sb = pw.tile([128, 16], F32, tag="b1")
                    b2_sb = pw.tile([128, 1], F32, tag="b2")
                    nc.sync.dma_start(w1_sb[:], W1[nlo : nlo + 128, :])
                    nc.sync.dma_start(w2_sb[:], W2[nlo : nlo + 128, :])
                    nc.sync.dma_start(b1_sb[:], B1[nlo : nlo + 128, :])
                    nc.sync.dma_start(b2_sb[:], B2[nlo : nlo + 128, :])

                    d1 = pd.tile([128, 48, 128], F32R, tag="d1")
                    d2 = pd.tile([128, 48, 128], F32R, tag="d2")
                    for jk in range(48):
                        nc.vector.tensor_scalar_mul(
                            d1[:, jk, :], ident[:], w1_sb[:, jk : jk + 1]
                        )
                        nc.vector.tensor_scalar_mul(
                            d2[:, jk, :], ident[:], w2_sb[:, jk : jk + 1]
                        )

                    for bp in range(B // 2):
                        b0 = bp * 2
                        # two batches side by side: N=476 >= 256 keeps the
                        # f32r matmuls at 1 cycle/row
                        x_t = px.tile([128, 2, L], F32R, tag="x")
                        nc.sync.dma_start(
                            x_t[:],
                            Xs[b0 : b0 + 2, nlo : nlo + 128, :].rearrange(
                                "b p t -> p b t"
                            ),
                        )

                        # conv1 -> h1 [128, 2, 16, 238]
                        h1_t = pa.tile([128, 2, 16, L1], F32R, tag="h1")
                        for j in range(16):
                            ps1 = ps_c1.tile([128, 2, L1], F32, tag="c1")
                            for k in range(KW):
                                nc.tensor.matmul(
                                    ps1[:],
                                    d1[:, j * KW + k, :],
                                    x_t[:, :, k : k + L1],
                                    start=(k == 0),
                                    stop=(k == KW - 1),
                                )
                            nc.scalar.activation(
                                h1_t[:, :, j, :],
                                ps1[:],
                                AF.Lrelu,
                                bias=b1_sb[:, j : j + 1],
                                alpha=0.01,
                            )

                        # conv2 -> h [128, 2, 236]
                        ps2 = ps_c2.tile([128, 2, L2], F32, tag="c2")
                        for j in range(16):
                            for k in range(KW):
                                nc.tensor.matmul(
                                    ps2[:],
                                    d2[:, j * KW + k, :],
                                    h1_t[:, :, j, k : k + L2],
                                    start=(j == 0 and k == 0),
                                    stop=(j == 15 and k == KW - 1),
                                )
                        h_t = pa.tile([128, 2, L2], F32R, tag="h")
                        nc.scalar.activation(
                            h_t[:],
                            ps2[:],
                            AF.Lrelu,
                            bias=b2_sb[:, 0:1],
                            alpha=0.01,
                        )

                        for bi in range(2):
                            b = b0 + bi
                            # transpose h -> hT [t', n] chunks (pad 236->256)
                            pta = ps_tr.tile([128, 256], F32R, tag="mt")
                            ptb = ps_tr.tile([128, 256], F32R, tag="mt")
                            nc.tensor.transpose(
                                pta[:, 0:128], h_t[:, bi, 0:128], ident[:]
                            )
                            nc.tensor.transpose(
                                ptb[: L2 - 128, 0:128], h_t[:, bi, 128:L2], ident[:]
                            )
                            hT_t = pa.tile([128, 2, 128], F32R, tag="hT")
                            nc.vector.tensor_copy(hT_t[:, 0, :], pta[:, 0:128])
                            nc.sync.dma_start(hT_t[:, 1, :], ZZ[:])
                            nc.vector.tensor_copy(
                                hT_t[: L2 - 128, 1, :], ptb[: L2 - 128, 0:128]
                            )

                            # xw = h @ Wg  -> [128 nodes, C]
                            psxw = ps_misc.tile([128, 256], F32, tag="m")
                            nc.tensor.matmul(
                                psxw[:, 0:C],
                                hT_t[:, 0, :],
                                wg0[:],
                                start=True,
                                stop=False,
                            )
                            nc.tensor.matmul(
                                psxw[:, 0:C],
                                hT_t[:, 1, :],
                                wg1[:],
                                start=False,
                                stop=True,
                            )
                            xw_sb = pa.tile([128, C], F32R, tag="xw")
                            nc.vector.tensor_copy(xw_sb[:], psxw[:, 0:C])
                            nc.sync.dma_start(cc_in[nlo : nlo + 128, b, :], xw_sb[:])
                            if DEBUG and nt == 0 and b == 0:
                                nc.sync.dma_start(dbg_h1[:], h1_t[:, 0, 0, :])
                                nc.sync.dma_start(dbg_h[:], h_t[:, 0, :])
                                nc.sync.dma_start(dbg_xw[:], xw_sb[:])

                            # lin1 = X @ L1W -> keep in lin1_all
                            xT_t = px.tile([128, 2, 128], F32R, tag="xT")
                            nc.sync.dma_start(
                                xT_t[:],
                                XTs[b, :, nlo : nlo + 128].rearrange(
                                    "(q p) n -> p q n", p=128
                                ),
                            )
                            pslin = ps_misc.tile([128, 256], F32, tag="m")
                            nc.tensor.matmul(
                                pslin[:, 0:C],
                                xT_t[:, 0, :],
                                l1w0[:],
                                start=True,
                                stop=False,
                            )
                            nc.tensor.matmul(
                                pslin[:, 0:C],
                                xT_t[:, 1, :],
                                l1w1[:],
                                start=False,
                                stop=True,
                            )
                            nc.vector.tensor_copy(
                                lin1_all[:, nt * B + b, :], pslin[:, 0:C]
                            )
                            if DEBUG and nt == 0 and b == 0:
                                nc.sync.dma_start(dbg_lin[:], lin1_all[:, 0, :])

                    # gather this node tile's xw across cores while the next
                    # tile's convs run
                    nc.gpsimd.collective_compute(
                        "AllGather",
                        OP.bypass,
                        replica_groups=[list(range(NCORES))],
                        ins=[cc_in[nlo : nlo + 128, :, :].opt()],
                        outs=[cc_outs[nt].opt()],
                    )

            # ---------------- Phase C1: aggregate + graphnorm + elu ------
            with tc.tile_pool(name="pc_at", bufs=3) as pat, tc.tile_pool(
                name="pc_rhs", bufs=3
            ) as prhs, tc.tile_pool(name="pc_t", bufs=2) as ptp, tc.tile_pool(
                name="ps_agg", bufs=4, space="PSUM"
            ) as ps_agg:
                aggps = [
                    ps_agg.tile([128, B, C], F32, tag="agg", name=f"agg{i}")
                    for i in range(NT)
                ]
                for st in range(ST):
                    slo = st * 128
                    at_t = pat.tile([128, NSH], F32R, tag="at")
                    nc.sync.dma_start(at_t[:], AT[slo : slo + 128, :])
                    xwf_t = prhs.tile([128, B * C], F32R, tag="xwf")
                    src_core, src_nt = st // NT, st % NT
                    nc.sync.dma_start(
                        xwf_t[:],
                        cc_outs[src_nt][
                            src_core * 128 : (src_core + 1) * 128, :, :
                        ].rearrange("p b c -> p (b c)"),
                    )
                    for nt2 in range(NT):
                        for half in range(2):
                            nc.tensor.matmul(
                                aggps[nt2][:, half * 2 : (half + 1) * 2, :],
                                at_t[:, nt2 * 128 : (nt2 + 1) * 128],
                                xwf_t[:, half * 512 : (half + 1) * 512],
                                start=(st == 0),
                                stop=(st == ST - 1),
                            )

                for nt2 in range(NT):
                    ps = aggps[nt2]
                    T_t = ptp.tile([128, B, C], F32, tag="T")
                    bgcn_b = cb[:, None, 0:C].to_broadcast((128, B, C))
                    cbias_b = cb[:, None, 3 * C : 4 * C].to_broadcast((128, B, C))
                    nc.vector.tensor_tensor(T_t[:], ps[:], bgcn_b, OP.add)
                    if DEBUG and nt2 == 0:
                        nc.sync.dma_start(dbg_T[:], T_t[:, 0, :])
                    m01 = ptp.tile([128, C], F32, tag="m01")
                    m23 = ptp.tile([128, C], F32, tag="m23")
                    msum = ptp.tile([128, C], F32, tag="msum")
                    nc.vector.tensor_tensor(m01[:], T_t[:, 0, :], T_t[:, 1, :], OP.add)
                    nc.vector.tensor_tensor(m23[:], T_t[:, 2, :], T_t[:, 3, :], OP.add)
                    nc.vector.tensor_tensor(msum[:], m01[:], m23[:], OP.add)
                    m2 = ptp.tile([128, C], F32, tag="m2")
                    nc.vector.tensor_tensor(m2[:], msum[:], gmsq, OP.mult)
                    ctr = ptp.tile([128, B, C], F32, tag="ctr")
                    nc.vector.tensor_tensor(
                        ctr[:], T_t[:], m2[:, None, :].to_broadcast((128, B, C)),
                        OP.subtract,
                    )
                    sq = ptp.tile([128, B, C], F32, tag="sq")
                    nc.vector.tensor_tensor(sq[:], ctr[:], ctr[:], OP.mult)
                    v = ptp.tile([128, C], F32, tag="v")
                    nc.vector.tensor_tensor(v[:], sq[:, 0, :], sq[:, 1, :], OP.add)
                    nc.vector.tensor_tensor(v[:], v[:], sq[:, 2, :], OP.add)
                    nc.vector.tensor_tensor(v[:], v[:], sq[:, 3, :], OP.add)
                    # invstd = exp(-0.5 * ln(v/4 + eps))
                    lnv = ptp.tile([128, C], F32, tag="lnv")
                    nc.scalar.activation(
                        lnv[:], v[:], AF.Ln, bias=eps_col[:, 0:1], scale=0.25
                    )
                    inv = ptp.tile([128, C], F32, tag="inv")
                    nc.scalar.activation(inv[:], lnv[:], AF.Exp, scale=-0.5)
                    gwi = ptp.tile([128, C], F32, tag="gwi")
                    nc.vector.tensor_tensor(gwi[:], inv[:], gnw, OP.mult)
                    lin_slice = lin1_all[:, nt2 * B : (nt2 + 1) * B, :]
                    t0 = ptp.tile([128, B, C], F32, tag="t0")
                    nc.vector.tensor_tensor(
                        t0[:], ctr[:], gwi[:, None, :].to_broadcast((128, B, C)),
                        OP.mult,
                    )
                    nc.vector.tensor_tensor(t0[:], t0[:], lin_slice, OP.add)
                    nc.vector.tensor_tensor(t0[:], t0[:], cbias_b, OP.add)
                    # elu(x) = max(x,0) + exp(min(x,0)) - 1
                    r = ptp.tile([128, B, C], F32, tag="r")
                    mn = ptp.tile([128, B, C], F32, tag="mn")
                    nc.vector.tensor_scalar_max(r[:], t0[:], 0.0)
                    nc.vector.tensor_scalar_min(mn[:], t0[:], 0.0)
                    e = ptp.tile([128, B, C], F32, tag="e")
                    nc.scalar.activation(e[:], mn[:], AF.Exp)
                    nc.vector.scalar_tensor_tensor(
                        lin_slice, e[:], -1.0, r[:], OP.add, OP.add
                    )
                    if DEBUG and nt2 == 0:
                        nc.sync.dma_start(dbg_tf[:], lin1_all[:, nt2 * B, :])

            # ---------------- Phase C2: node-mean pool + classifier ------
            with tc.tile_pool(name="pc2", bufs=1) as p2, tc.tile_pool(
                name="ps_p", bufs=1, space="PSUM"
            ) as ps_p:
                pspool = ps_p.tile([128, 2 * B], F32, tag="pool")
                for b in range(B):
                    for cq in range(2):
                        for nt2 in range(NT):
                            nc.tensor.matmul(
                                pspool[:, cq * B + b : cq * B + b + 1],
                                lin1_all[:, nt2 * B + b, cq * 128 : (cq + 1) * 128],
                                ones_col[:],
                                start=(nt2 == 0),
                                stop=(nt2 == NT - 1),
                            )
                poolv = p2.tile([128, 2 * B], F32, tag="poolv")
                nc.vector.tensor_copy(poolv[:], pspool[:])
                nc.sync.dma_start(cc2_in[:], poolv[:])
                if DEBUG:
                    nc.sync.dma_start(dbg_pv[:], poolv[:])
                nc.gpsimd.collective_compute(
                    "AllReduce",
                    OP.add,
                    replica_groups=[list(range(NCORES))],
                    ins=[cc2_in.opt()],
                    outs=[cc2_out.opt()],
                )
                redv = p2.tile([128, 2 * B], F32, tag="redv")
                nc.sync.dma_start(redv[:], cc2_out[:])
                nc.vector.tensor_scalar_mul(redv[:], redv[:], 1.0 / N)
                nc.sync.dma_start(out_pool[:], redv[:])
                if DEBUG:
                    nc.sync.dma_start(dbg_rv[:], redv[:])

                psc = ps_p.tile([16, B], F32, tag="cls")
                nc.tensor.matmul(
                    psc[:], outw[:, 0, :], redv[:, 0:B], start=True, stop=False
                )
                nc.tensor.matmul(
                    psc[:], outw[:, 1, :], redv[:, B : 2 * B], start=False, stop=True
                )
                cls_sb = p2.tile([16, B], F32, tag="clssb")
                nc.vector.tensor_scalar_add(cls_sb[:], psc[:], outb[:, 0:1])
                nc.sync.dma_start(out_cls[:], cls_sb[:])

    _split_sync_waits(nc)
    _built = nc
    return nc


def _host_prep(inputs):
    X = np.asarray(inputs["X"], np.float32)
    edge_index = np.asarray(inputs["edge_index"])
    w1 = np.asarray(inputs["w1"], np.float32)
    b1 = np.asarray(inputs["b1"], np.float32)
    w2 = np.asarray(inputs["w2"], np.float32)
    b2 = np.asarray(inputs["b2"], np.float32)
    W_gcn = np.asarray(inputs["W_gcn"], np.float32)
    b_gcn = np.asarray(inputs["b_gcn"], np.float32)
    gn_weight = np.asarray(inputs["gn_weight"], np.float32)
    gn_bias = np.asarray(inputs["gn_bias"], np.float32)
    gn_mean_scale = np.asarray(inputs["gn_mean_scale"], np.float32)
    lin1_W = np.asarray(inputs["lin1_W"], np.float32)
    lin1_b = np.asarray(inputs["lin1_b"], np.float32)
    out_W = np.asarray(inputs["out_W"], np.float32)
    out_b = np.asarray(inputs["out_b"], np.float32)

    src = edge_index[0].astype(np.int64)
    dst = edge_index[1].astype(np.int64)
    deg = np.bincount(dst, minlength=N).astype(np.float32) + 1.0
    dinv = 1.0 / np.sqrt(deg)
    norm = (dinv[src] * dinv[dst]).astype(np.float32)

    AT_full = np.zeros((N, N), np.float32)
    np.add.at(AT_full, (src, dst), norm)
    AT_full[np.arange(N), np.arange(N)] += dinv * dinv

    Xt = np.ascontiguousarray(X.transpose(0, 2, 1))  # [B, L, N]

    Wg_pad = np.zeros((256, C), np.float32)
    Wg_pad[:L2] = W_gcn
    L1W_pad = np.zeros((256, C), np.float32)
    L1W_pad[:L] = lin1_W
    cv = np.broadcast_to(
        np.concatenate(
            [b_gcn, gn_mean_scale * 0.25, gn_weight, gn_bias + lin1_b]
        ).astype(np.float32)[None, :],
        (128, 4 * C),
    ).copy()
    ident = np.eye(128, dtype=np.float32)

    w1n = w1.reshape(N, 16, KW)
    b1n = b1.reshape(N, 16)
    w2n = w2.reshape(N, 16, KW)

    in_maps = []
    for i in range(NCORES):
        lo, hi = i * NSH, (i + 1) * NSH
        XTs_i = np.zeros((B, 256, NSH), np.float32)
        XTs_i[:, :L, :] = Xt[:, :, lo:hi]
        in_maps.append(
            {
                "Xs": np.ascontiguousarray(X[:, lo:hi, :]),
                "XTs": XTs_i,
                "AT": np.ascontiguousarray(AT_full[:, lo:hi]),
                "W1": np.ascontiguousarray(w1n[lo:hi].reshape(NSH, 48)),
                "B1": np.ascontiguousarray(b1n[lo:hi]),
                "W2": np.ascontiguousarray(w2n[lo:hi].reshape(NSH, 48)),
                "B2": np.ascontiguousarray(b2[lo:hi, None]),
                "Wg": Wg_pad,
                "L1W": L1W_pad,
                "CV": cv,
                "OW": out_W,
                "OB": out_b[:, None].astype(np.float32),
                "IDN": ident,
                "ZZ": np.zeros((128, 128), np.float32),
            }
        )
    return in_maps


def kernel(**inputs):
    nc = _build()
    in_maps = _host_prep(inputs)
    res = run_bass_kernel_spmd(nc, in_maps, list(range(NCORES)))
    op = res.results[0]["out_pool"]  # [128, 2B]  (c_low, cq*B + b)
    oc = res.results[0]["out_cls"]  # [16, B]
    output = np.empty((B, C), np.float32)
    for cq in range(2):
        for b in range(B):
            output[b, cq * 128 : (cq + 1) * 128] = op[:, cq * B + b]
    class_output = np.ascontiguousarray(oc.T)
    return (output, class_output)
